# revision 61
# baseline (speedup 1.0000x reference)
"""Multi-head attention (B=2, S=2048, E=768, H=12, D=64) on 8 NeuronCores.

Sharding: core c -> batch b = c//4, head group hg = c%4 (3 heads each).
Each core computes the qkv projection for its 3 heads, attention, and a
partial output projection (rows of w_proj for its heads). Host sums the
partials per batch and adds the bias terms (tensor-parallel unshard).

Device dataflow (everything transposed so no on-chip transposes are needed,
and every matmul has a full K=128 contraction — K=64 matmuls run the PE at
half clock):
  xT [768, 2048]  (host-pretransposed, fp16), loaded in column halves so
           the projection's partial-K PSUM groups start during the DMA
           stream; ~30 warmup matmuls on a dummy tile keep the PE clock
           ramping while the DMA streams.
  qkT[t] = (w_qk_tile_t)^T @ xT -> [128, 2048] tiles t=0..4 with w columns
           laid out [q0|q1], [k0|0], [0|k1], [0|q2], [0|k2]: each head's
           scoresT matmul then uses a full-128-partition stationary whose
           zero half kills the other head's rows.  The q columns of w_qkv
           (and q biases) are host-prescaled by 1024*log2(e)/8 so the
           scores PSUM is directly in fp16-Schraudolph J units.
  v'   = xT_tile^T @ w_v -> per-head per-Sk-block [128, 128] blocks:
           cols 0:64 = 1.0, cols 64:128 = v, so the AV matmul produces the
           softmax denominator in PSUM rows 0:64 and values in 64:128.
  pT   = exp(scores) computed split across two engines per scores step:
           g=0 -> ScalarE table exp (scale=1/(1024*log2 e)), fp16 out;
           g=1 -> VectorE one-op Schraudolph: int16(psum + (15360+delta)),
           written through a bitcast AP into the same fp16 pt tile.  Both
           produce ~e^s; numerator and denominator use the same values so
           the approximation error largely cancels in the softmax ratio.
  avT  = v'^T @ pT          -> [128, 512] PSUM per (head, Sq-chunk), with
           the matmul stream lag-emitted behind the exp stream.
  outT = av[64:128] * approx(1/av[0:64])  (DVE)
  y    = w_projA^T @ st01 + w_projB^T @ outT2 fused into ONE PSUM
           accumulation group per output chunk (single DRAM output, half
           the store traffic of separate yA/yB).  All y work sits in the
           tail; h2's AV chunks nq0/1 are staged through SBUF partials so
           only one matmul + adds + normalize separate the final exp from
           the y projections.
"""

import threading

import numpy as np

import concourse.bass as bass
import concourse.tile as tile
from concourse import bacc, mybir
from concourse.bass import ts, ds
from concourse.bass_utils import run_bass_kernel_spmd

F32 = mybir.dt.float32
F16 = mybir.dt.float16
I16 = mybir.dt.int16

EMBED = 768
NH = 12
D = 64
B = 2
S = 2048
HPC = 3          # heads per core
NCORES = 8
P = 128
KC = EMBED // P  # 6 contraction chunks
NQ = S // 512    # 4 Sq chunks of 512
NSK = S // P     # 16 Sk blocks
NT = 5           # qk projection tiles

# fp16 Schraudolph exp: J = s_true * 1024*log2(e) + (15360 + DELTA);
# bitcast(int16(J)) ~= e^s.  The 1024*log2(e)/8 factor is folded into the
# host-side q weights/biases so the scores PSUM is already in J units.
JSCALE = 1024.0 * np.log2(np.e)          # 1477.3195
JBIAS = 15360.0 - 45.0                   # delta=-45 balances the sawtooth
ACT_SCALE = float(1.0 / JSCALE)          # ScalarE exp reads the same psum

N_WARMUP = 26    # dummy matmuls that ramp the PE clock during the DMA-in


def _build_kernel(nc):
    # weights arrive host-packed [128, n] so each is one wide-descriptor DMA
    xT = nc.dram_tensor("xT", [EMBED, S], F16, kind="ExternalInput").ap()
    wqk = nc.dram_tensor("w_qk", [P, KC * 3 * P], F16, kind="ExternalInput").ap()
    b2d = nc.dram_tensor("b2", [P, 2], F32, kind="ExternalInput").ap()
    wv = nc.dram_tensor("w_v", [P, KC * HPC * D], F16, kind="ExternalInput").ap()
    wp = nc.dram_tensor("w_p", [P, 2 * EMBED], F16, kind="ExternalInput").ap()
    idn = nc.dram_tensor("ident", [P, P], F16, kind="ExternalInput").ap()
    # y stored in staging layout [nq*3+mt2, 128, 1024]: each drained SBUF
    # tile DMAs out as one contiguous-per-partition block (host reshuffles)
    y = nc.dram_tensor("y", [NQ * 3, P, 1024], F16, kind="ExternalOutput").ap()

    with tile.TileContext(nc) as tc:
        with (
            tc.tile_pool(name="wpool", bufs=1) as wpool,
            tc.tile_pool(name="xpool", bufs=1) as xpool,
            tc.tile_pool(name="qkpool", bufs=1) as qkpool,
            tc.tile_pool(name="vpool", bufs=1) as vpool,
            tc.tile_pool(name="ptpool", bufs=24) as ptpool,
            tc.tile_pool(name="opool", bufs=1) as opool,
            tc.tile_pool(name="rlpool", bufs=4) as rlpool,
            tc.tile_pool(name="psum", bufs=3, space="PSUM") as psum,
        ):
            # ---- PE warmup: dummy matmuls keep the PE busy (and its clock
            # ramping) from its ~7.5us engine-init floor while the DMAs
            # stream.
            dummy = wpool.tile([P, 512], F16, name="dummy")
            nc.vector.memset(dummy, 0.0)
            wps = psum.tile([P, 512], F32, tag="av", bufs=2, name="ps_warm")
            for _ in range(N_WARMUP):
                nc.tensor.matmul(wps, lhsT=dummy[:, 0:P], rhs=dummy,
                                 start=True, stop=True)

            # ---- loads.  The weights come host-packed as [128, n] tiles so
            # each is ONE dma_start with wide descriptors; the xT tiles are
            # row-split in two and the dma_starts spread across three
            # issuing engines (descriptor generation costs ~0.6us of the
            # issuing engine per dma_start, so one engine issuing them all
            # would serialize the stream).
            wqk_big = wpool.tile([P, KC * 3 * P], F16, name="wqk")
            nc.sync.dma_start(out=wqk_big[0:64, :], in_=wqk[0:64, :])
            nc.scalar.dma_start(out=wqk_big[64:P, :], in_=wqk[64:P, :])
            wqk_t = [wqk_big[:, ds(k * 3 * P, 3 * P)] for k in range(KC)]
            xT_t = []
            dma_engs = [nc.sync, nc.gpsimd, nc.scalar]
            for k in range(KC):
                xT_k = xpool.tile([P, S], F16, name=f"xT{k}")
                for r in range(2):
                    dma_engs[(2 * k + r) % 3].dma_start(
                        out=xT_k[ts(r, 64), :], in_=xT[ds(k * P + r * 64, 64), :]
                    )
                xT_t.append(xT_k)
            b2 = wpool.tile([P, 2], F32, name="b2")
            nc.sync.dma_start(out=b2, in_=b2d)
            wv_big = wpool.tile([P, KC * HPC * D], F16, name="wv")
            nc.gpsimd.dma_start(out=wv_big, in_=wv)
            wv_t = [wv_big[:, ds(k * HPC * D, HPC * D)] for k in range(KC)]
            wp_big = wpool.tile([P, 2 * EMBED], F16, name="wp")
            nc.sync.dma_start(out=wp_big, in_=wp)
            wpA = wp_big[:, 0:EMBED]
            wpB = wp_big[:, EMBED : 2 * EMBED]
            ident = wpool.tile([P, P], F16, name="ident")
            nc.gpsimd.dma_start(out=ident, in_=idn)

            # ---- persistent sbuf tensors + memsets (gpsimd, idle early)
            qkT = {
                t: qkpool.tile([P, S], F16, name=f"qkT{t}", tag=f"qkT{t}")
                for t in range(NT)
            }
            for t in (1, 2, 3, 4):
                zero = slice(D, P) if t == 1 else slice(0, D)
                nc.gpsimd.memset(qkT[t][zero, :], 0.0)
            vp = []
            for h in range(HPC):
                vp_h = vpool.tile([P, NSK * P], F16, name=f"vp{h}", tag=f"vp{h}")
                nc.gpsimd.memset(
                    vp_h.rearrange("p (s c) -> p s c", c=P)[:, :, 0:D], 1.0
                )
                vp.append(vp_h)
            st01 = opool.tile([P, S], F16)   # heads 0 (rows 0:64) and 1 (64:128)
            outT2 = opool.tile([P, S], F16)  # head 2 (rows 0:64; 64:128 zeroed)
            nc.gpsimd.memset(outT2[D:P, :], 0.0)

            # warm the Exp table while the loads stream
            wexp = rlpool.tile([P, 1], F32, name="wexp", tag="wexp", bufs=1)
            nc.scalar.activation(
                out=wexp, in_=dummy[:, 0:1],
                func=mybir.ActivationFunctionType.Exp, scale=1.0,
            )

            # ---- early qk projection: 8 partial-K PSUM groups fed as the
            # xT tiles land; left-half (Sq 0:1024) groups emitted first so
            # they only depend on the half-0 DMAs.
            scA = psum.tile([P, 1024], F32, tag="sc", name="scA")
            scB = psum.tile([P, 1024], F32, tag="sc", name="scB")
            scC = psum.tile([P, 1024], F32, tag="sc", name="scC")
            avA = psum.tile([P, 512], F32, tag="av", bufs=2, name="avA")
            avB = psum.tile([P, 512], F32, tag="av", bufs=2, name="avB")
            egroups = [
                (scA[:, 0:512], 0, 0),
                (scA[:, 512:1024], 0, 1),
                (scC[:, 0:512], 1, 0),
                (scC[:, 512:1024], 1, 1),
                (scB[:, 0:512], 0, 2),
                (scB[:, 512:1024], 0, 3),
                (avA, 1, 2),
                (avB, 1, 3),
            ]
            for k in range(KC):
                for dst, t, nq in egroups:
                    nc.tensor.matmul(
                        dst,
                        lhsT=wqk_t[k][:, ts(t, P)],
                        rhs=xT_t[k][:, ts(nq, 512)],
                        start=(k == 0),
                        stop=(k == KC - 1),
                    )
            # drains: the h0-critical pieces (qkT0, qkT1) go on the DVE, the
            # h1-only qkT2 copies on ScalarE, so neither engine runs 6 deep
            # when the first g1 exps arrive.
            nc.scalar.activation(
                out=qkT[0][:, 0:1024], in_=scA,
                func=mybir.ActivationFunctionType.Identity, bias=b2[:, 0:1],
            )
            nc.vector.tensor_copy(out=qkT[1][0:D, 0:1024], in_=scC[0:D, :])
            nc.scalar.activation(
                out=qkT[0][:, 1024:2048], in_=scB,
                func=mybir.ActivationFunctionType.Identity, bias=b2[:, 0:1],
            )
            nc.vector.tensor_copy(out=qkT[1][0:D, ts(2, 512)], in_=avA[0:D, :])
            nc.vector.tensor_copy(out=qkT[1][0:D, ts(3, 512)], in_=avB[0:D, :])
            nc.scalar.copy(out=qkT[2][D:P, 0:1024], in_=scC[D:P, :])
            nc.scalar.copy(out=qkT[2][D:P, ts(2, 512)], in_=avA[D:P, :])
            nc.scalar.copy(out=qkT[2][D:P, ts(3, 512)], in_=avB[D:P, :])

            # ---- kernel building blocks
            def scores_step(kt, qt, pts, act_both=False):
                """One Sk block of scores for one head: 4 matmuls + 2 exps,
                one on ScalarE (table) and one on VectorE (Schraudolph).
                act_both puts both on ScalarE — used for the last h2 steps
                so the DVE enters the tail with an empty queue."""
                sk = len(pts)
                pt = ptpool.tile([P, S], F16, name="pt", tag="pt")
                pts.append(pt)
                for g in range(2):
                    sps = psum.tile([P, 1024], F32, tag="sc", name="ps_s")
                    for j in range(2):
                        nc.tensor.matmul(
                            sps[:, ts(j, 512)],
                            lhsT=kt[:, ts(sk, P)],
                            rhs=qt[:, ds(g * 1024 + j * 512, 512)],
                            start=True,
                            stop=True,
                        )
                    if g == 1 and not act_both:
                        nc.vector.tensor_scalar_add(
                            out=pt[:, ts(g, 1024)].bitcast(I16),
                            in0=sps,
                            scalar1=JBIAS,
                        )
                    else:
                        nc.scalar.activation(
                            out=pt[:, ts(g, 1024)],
                            in_=sps,
                            func=mybir.ActivationFunctionType.Exp,
                            scale=ACT_SCALE,
                        )

            def qk2_chunk(n01):
                """tile2 = [q2|k2] projection for Sq chunks n01, n01+1:
                q2 -> qkT3 rows 64:128, k2 -> qkT4 rows 64:128."""
                sc = psum.tile([P, 1024], F32, tag="sc", name="ps_qk2")
                for k in range(KC):
                    for j in range(2):
                        nc.tensor.matmul(
                            sc[:, ts(j, 512)],
                            lhsT=wqk_t[k][:, ts(2, P)],
                            rhs=xT_t[k][:, ts(n01 + j, 512)],
                            start=(k == 0),
                            stop=(k == KC - 1),
                        )
                nc.vector.tensor_scalar_add(
                    out=qkT[3][D:P, ds(n01 * 512, 1024)],
                    in0=sc[0:D, :],
                    scalar1=b2[D:P, 1:2],
                )
                nc.scalar.copy(
                    out=qkT[4][D:P, ds(n01 * 512, 1024)], in_=sc[D:P, :]
                )

            def v_super(g4):
                """v projection for Sk blocks 4*g4 .. 4*g4+3 in one sc slot;
                one strided drain copy per head (ScalarE: DVE is on exp duty)."""
                # 256-col pitch per group: a 192-wide accumulation group must
                # not straddle a 512-float PSUM bank boundary
                vps = psum.tile([P, 1024], F32, tag="sc", name="ps_v")
                for j in range(4):
                    st = 4 * g4 + j
                    for k in range(KC):
                        nc.tensor.matmul(
                            vps[:, ds(j * 256, HPC * D)],
                            lhsT=xT_t[k][:, ts(st, P)],
                            rhs=wv_t[k],
                            start=(k == 0),
                            stop=(k == KC - 1),
                        )
                vv = vps.rearrange("p (j r) -> p j r", r=256)
                for h in range(HPC):
                    nc.scalar.copy(
                        out=vp[h].rearrange("p (s c) -> p s c", c=P)[
                            :, ds(4 * g4, 4), D:P
                        ],
                        in_=vv[:, :, ds(h * D, D)],
                    )

            def norm_chunk(h, nq, avp):
                rr = rlpool.tile([D, 512], F32, name="rr", tag="rr")
                nc.vector.reciprocal_approx_fast(out=rr, in_=avp[0:D, :])
                if h == 0:
                    dst = st01[0:D, ts(nq, 512)]
                elif h == 1:
                    dst = st01[D:P, ts(nq, 512)]
                else:
                    dst = outT2[0:D, ts(nq, 512)]
                nc.vector.tensor_mul(out=dst, in0=avp[D:P, :], in1=rr)

            class AvChunk:
                """Lag-emitted AV accumulation: emit() is called once per
                scores step so the matmul stream trails the exp stream."""

                def __init__(self, h, pts, nq, nsk, finish):
                    self.h, self.pts, self.nq, self.nsk, self.finish = (
                        h, pts, nq, nsk, finish)
                    self.av = psum.tile([P, 512], F32, tag="av", bufs=2, name="ps_av")
                    self.j = 0

                def emit(self, upto, cap=NSK):
                    upto = min(upto, self.j + cap)
                    while self.j < min(upto, self.nsk):
                        j = self.j
                        nc.tensor.matmul(
                            self.av,
                            lhsT=vp[self.h][:, ts(j, P)],
                            rhs=self.pts[j][:, ts(self.nq, 512)],
                            start=(j == 0),
                            stop=(j == self.nsk - 1),
                        )
                        self.j += 1
                    if self.j == self.nsk:
                        self.j += 1
                        self.finish(self.av)

            def av_burst(h, pts, nq):
                c = AvChunk(h, pts, nq, NSK, lambda av: norm_chunk(h, nq, av))
                c.emit(NSK)

            a2sb = {}

            def av2a_stage(nq, av):
                # fp16 staging via ScalarE: the tail re-injects it into PSUM
                # with an identity matmul, so no DVE adds are needed
                sb = rlpool.tile([P, 512], F16, name="a2s", tag="a2s", bufs=2)
                nc.scalar.copy(out=sb, in_=av)
                a2sb[nq] = sb

            def y_a(nq):
                """wpA half of the fused output projection: depends only on
                st01, so it can be emitted while the DVE still normalizes
                the outT2 chunk the wpB half needs."""
                tiles = [psum.tile([P, 1024], F32, tag="sc", name="ps_y")
                         for _ in range(3)]
                for mt2 in range(3):
                    for j in range(2):
                        nc.tensor.matmul(
                            tiles[mt2][:, ts(j, 512)],
                            lhsT=wpA[:, ts(2 * mt2 + j, P)],
                            rhs=st01[:, ts(nq, 512)],
                            start=True,
                            stop=False,
                        )
                return tiles

            def y_b(nq, tiles, last=False):
                """wpB half, tile-by-tile with drain+DMA interleaved.  The
                very last tile of the kernel drains as two halves on both
                engines with two DMA queues, shortening the final flush."""
                for mt2 in range(3):
                    for j in range(2):
                        nc.tensor.matmul(
                            tiles[mt2][:, ts(j, 512)],
                            lhsT=wpB[:, ts(2 * mt2 + j, P)],
                            rhs=outT2[:, ts(nq, 512)],
                            start=False,
                            stop=True,
                        )
                    ysb = rlpool.tile([P, 1024], F16, name="ysb", tag="ysb", bufs=4)
                    i = nq * 3 + mt2
                    if mt2 == 1:
                        nc.vector.tensor_copy(out=ysb, in_=tiles[mt2])
                    else:
                        nc.scalar.copy(out=ysb, in_=tiles[mt2])
                    (nc.gpsimd if mt2 % 2 else nc.sync).dma_start(out=y[i], in_=ysb)

            def y_chunk(nq, last=False):
                y_b(nq, y_a(nq), last)

            # ================= emission schedule =================
            pts0, pts1, pts2 = [], [], []

            # ---- h0 scores; fillers: remaining qk/v chunks + lagged av0
            lagged = []
            fill0 = {
                1: lambda: v_super(0),
                3: lambda: qk2_chunk(0),
                5: lambda: v_super(1),
                7: lambda: qk2_chunk(2),
                9: lambda: v_super(2),
                11: lambda: v_super(3),
            }
            for sk in range(NSK):
                scores_step(qkT[1], qkT[0], pts0)
                if sk in fill0:
                    fill0[sk]()
                if sk == 2:
                    lagged.append(AvChunk(0, pts0, 0, NSK,
                                          lambda av: norm_chunk(0, 0, av)))
                if sk == 4:
                    lagged.append(AvChunk(0, pts0, 1, NSK,
                                          lambda av: norm_chunk(0, 1, av)))
                for c in lagged:
                    c.emit(sk - 1)

            # ---- h1 scores; fillers: av0 tail bursts + lagged av1
            fast = []
            for sk in range(NSK):
                scores_step(qkT[2], qkT[0], pts1)
                if sk == 0:
                    for c in lagged:
                        c.emit(NSK)
                    lagged = []
                    fast.append(AvChunk(0, pts0, 2, NSK,
                                        lambda av: norm_chunk(0, 2, av)))
                if sk == 4:
                    fast.append(AvChunk(0, pts0, 3, NSK,
                                        lambda av: norm_chunk(0, 3, av)))
                for c in fast:
                    c.emit(NSK, cap=4)
                if sk == 2:
                    lagged.append(AvChunk(1, pts1, 0, NSK,
                                          lambda av: norm_chunk(1, 0, av)))
                if sk == 4:
                    lagged.append(AvChunk(1, pts1, 1, NSK,
                                          lambda av: norm_chunk(1, 1, av)))
                for c in lagged:
                    c.emit(sk - 1)

            # ---- h2 scores; fillers: av1 tail bursts, staged av2 (nq0/1
            # through SBUF partials) and natural-lag av2 chunks (nq2/3)
            fast = []
            av23 = []
            for sk in range(NSK):
                scores_step(qkT[4], qkT[3], pts2, act_both=(sk >= NSK - 2))
                if sk == 0:
                    for c in lagged:
                        c.emit(NSK)
                    lagged = []
                    fast.append(AvChunk(1, pts1, 2, NSK,
                                        lambda av: norm_chunk(1, 2, av)))
                if sk == 4:
                    fast.append(AvChunk(1, pts1, 3, NSK,
                                        lambda av: norm_chunk(1, 3, av)))
                for c in fast:
                    c.emit(NSK, cap=4)
                if sk == 2:
                    lagged.append(AvChunk(2, pts2, 0, NSK - 1,
                                          lambda av: av2a_stage(0, av)))
                if sk == 4:
                    lagged.append(AvChunk(2, pts2, 1, NSK - 1,
                                          lambda av: av2a_stage(1, av)))
                if sk == 8:
                    av23.append(AvChunk(2, pts2, 2, NSK, None))
                    av23[-1].finish = lambda av, c=av23[-1]: setattr(c, "done", av)
                if sk == 10:
                    av23.append(AvChunk(2, pts2, 3, NSK, None))
                    av23[-1].finish = lambda av, c=av23[-1]: setattr(c, "done", av)
                for c in lagged:
                    c.emit(min(sk - 1, NSK - 3))
                for c in av23:
                    c.emit(min(sk - 1, NSK - 2), cap=4)
            for c in lagged:
                c.emit(NSK - 1)

            # ---- tail: nq2/3 finish with their sk15 matmul + normalize;
            # nq0/1 via the staged partial + one matmul + adds; the fused
            # y chunks run on the PE while the DVE normalizes ahead.
            def av2b_mm(nq):
                """Re-inject the fp16 staged partial into PSUM (identity
                matmul) and accumulate the sk15 block on top: the finish is
                then a plain recip+mul like the other chunks."""
                av2b = psum.tile([P, 512], F32, tag="av", bufs=2, name="ps_a2b")
                nc.tensor.matmul(
                    av2b, lhsT=ident, rhs=a2sb[nq], start=True, stop=False,
                )
                nc.tensor.matmul(
                    av2b,
                    lhsT=vp[2][:, ts(NSK - 1, P)],
                    rhs=pts2[NSK - 1][:, ts(nq, 512)],
                    start=False,
                    stop=True,
                )
                return av2b

            # Tail: y(2)'s wpA half first (needs only st01 — its PSUM slots
            # freed by the ScalarE exps of sk14/15), covering the final-exp
            # latency; then the sk15 AV matmuls and the DVE normalize chain
            # interleaved so each y chunk's outT2 lands just in time.
            t2 = y_a(2)
            av23[0].emit(NSK)                     # sk15 matmul, stashes psum
            av23[1].emit(NSK)
            norm_chunk(2, 2, av23[0].done)        # DVE: norm2
            a2b0 = av2b_mm(0)                     # reuses av23[0]'s slot
            norm_chunk(2, 3, av23[1].done)        # DVE: norm3
            y_b(2, t2)
            a2b1 = av2b_mm(1)
            norm_chunk(2, 0, a2b0)                # DVE: outT2 nq0 ready
            t3 = y_a(3)
            y_b(3, t3)
            norm_chunk(2, 1, a2b1)
            t0 = y_a(0)
            y_b(0, t0)
            y_chunk(1, last=True)
    return nc


_CACHE = threading.Lock(), {}


def _get_nc():
    lock, cache = _CACHE
    with lock:
        if "nc" not in cache:
            nc = bacc.Bacc("TRN2", target_bir_lowering=False, debug=False)
            _build_kernel(nc)
            nc.compile()
            cache["nc"] = nc
        return cache["nc"]


def _shard_inputs(x, w_qkv, b_qkv, w_proj):
    """Build the 8 per-core input maps (host-side sharding/layout)."""
    qscale = float(JSCALE) / 8.0   # fold the Schraudolph J scale into q
    in_maps = []
    for c in range(NCORES):
        b = c // 4
        hg = c % 4
        h0 = HPC * hg
        qc = [np.s_[D * (h0 + i) : D * (h0 + i + 1)] for i in range(HPC)]
        kc = [np.s_[EMBED + D * (h0 + i) : EMBED + D * (h0 + i + 1)] for i in range(HPC)]
        vc = [np.s_[2 * EMBED + D * (h0 + i) : 2 * EMBED + D * (h0 + i + 1)] for i in range(HPC)]

        # projected w tiles: [q0|q1], [k0|k1], [q2|k2]; q columns prescaled
        w_qk = np.zeros((EMBED, 3 * P), dtype=np.float32)
        halves = [
            (0, 0, qc[0], qscale), (0, 1, qc[1], qscale),
            (1, 0, kc[0], 1.0), (1, 1, kc[1], 1.0),
            (2, 0, qc[2], qscale), (2, 1, kc[2], 1.0),
        ]
        for t, half, cols, sc in halves:
            w_qk[:, t * P + half * D : t * P + half * D + D] = w_qkv[:, cols] * sc
        # q biases only: col 0 = [q0|q1], col 1 rows 64:128 = q2. The k
        # biases are constant over keys and cancel in the softmax; the v
        # bias is applied on the host.
        b2 = np.zeros((P, 2), dtype=np.float32)
        b2[0:D, 0] = b_qkv[qc[0]] * qscale
        b2[D:P, 0] = b_qkv[qc[1]] * qscale
        b2[D:P, 1] = b_qkv[qc[2]] * qscale

        w_v = np.concatenate([w_qkv[:, s] for s in vc], axis=1)
        # w_proj rows for these heads; B half zero-padded to K=128
        w_p = np.zeros((2 * P, EMBED), dtype=np.float32)
        w_p[0:P] = w_proj[D * h0 : D * h0 + P]
        w_p[P : P + D] = w_proj[D * h0 + P : D * (h0 + HPC)]
        # pack the [6*128, n] weight stacks as [128, 6*n] so each SBUF tile
        # loads with one wide-descriptor DMA
        w_qk_p = w_qk.reshape(KC, P, 3 * P).transpose(1, 0, 2).reshape(P, KC * 3 * P)
        w_v_p = w_v.reshape(KC, P, HPC * D).transpose(1, 0, 2).reshape(P, KC * HPC * D)
        w_p_p = w_p.reshape(2, P, EMBED).transpose(1, 0, 2).reshape(P, 2 * EMBED)
        in_maps.append(
            {
                "xT": np.ascontiguousarray(x[b].T).astype(np.float16),
                "w_qk": np.ascontiguousarray(w_qk_p).astype(np.float16),
                "b2": b2,
                "w_v": np.ascontiguousarray(w_v_p).astype(np.float16),
                "w_p": np.ascontiguousarray(w_p_p).astype(np.float16),
                "ident": np.eye(P, dtype=np.float16),
            }
        )
    return in_maps


def kernel(x, w_qkv, b_qkv, w_proj, b_proj, _results_hook=None):
    x = np.asarray(x, dtype=np.float32)
    w_qkv = np.asarray(w_qkv, dtype=np.float32)
    b_qkv = np.asarray(b_qkv, dtype=np.float32)
    w_proj = np.asarray(w_proj, dtype=np.float32)
    b_proj = np.asarray(b_proj, dtype=np.float32)

    nc = _get_nc()
    in_maps = _shard_inputs(x, w_qkv, b_qkv, w_proj)
    res = run_bass_kernel_spmd(nc, in_maps, core_ids=list(range(NCORES)))
    if _results_hook is not None:
        _results_hook(res)

    # unshard: sum the 4 head-group partials per batch, add bias terms.
    # y arrives in staging layout [nq*3+mt2, 128, 1024]; logical row
    # 2*mt2*128 + (c//512)*128 + p, col nq*512 + c%512.
    b_v = b_qkv[2 * EMBED :]
    bias_row = b_v @ w_proj + b_proj  # [768]
    out = np.empty((B, S, EMBED), dtype=np.float32)
    for b in range(B):
        acc = np.zeros((EMBED, S), dtype=np.float32)
        for hg in range(4):
            yst = res.results[4 * b + hg]["y"].astype(np.float32)
            yst = yst.reshape(NQ, 3, P, 2, 512)          # nq, mt2, p, half, c
            acc += (
                yst.transpose(1, 3, 2, 0, 4)             # mt2, half, p, nq, c
                .reshape(EMBED, S)
            )
        out[b] = acc.T + bias_row
    return out


# revision 63
# speedup vs baseline: 1.0075x; 1.0075x over previous
"""Multi-head attention (B=2, S=2048, E=768, H=12, D=64) on 8 NeuronCores.

Sharding: core c -> batch b = c//4, head group hg = c%4 (3 heads each).
Each core computes the qkv projection for its 3 heads, attention, and a
partial output projection (rows of w_proj for its heads). Host sums the
partials per batch and adds the bias terms (tensor-parallel unshard).

Device dataflow (everything transposed so no on-chip transposes are needed,
and every matmul has a full K=128 contraction — K=64 matmuls run the PE at
half clock):
  xT [768, 2048]  (host-pretransposed, fp16), row-split DMAs spread over
           three issuing engines; ~26 warmup matmuls on a dummy tile keep
           the PE busy from its ~7.5us engine-init floor while the DMA
           streams (weights arrive host-packed [128, n] so each loads
           with one wide-descriptor DMA).
  qkT[t] = (w_qk_tile_t)^T @ xT -> [128, 2048] tiles t=0..4 with w columns
           laid out [q0|q1], [k0|0], [0|k1], [0|q2], [0|k2]: each head's
           scoresT matmul then uses a full-128-partition stationary whose
           zero half kills the other head's rows.  The q columns of w_qkv
           (and q biases) are host-prescaled by 1024*log2(e)/8 so the
           scores PSUM is directly in fp16-Schraudolph J units.
  v'   = xT_tile^T @ w_v -> per-head per-Sk-block [128, 128] blocks:
           cols 0:64 = 1.0, cols 64:128 = v, so the AV matmul produces the
           softmax denominator in PSUM rows 0:64 and values in 64:128.
  pT   = exp(scores) computed split across two engines per scores step:
           g=0 -> ScalarE table exp (scale=1/(1024*log2 e)), fp16 out;
           g=1 -> VectorE one-op Schraudolph: int16(psum + (15360+delta)),
           written through a bitcast AP into the same fp16 pt tile.  Both
           produce ~e^s; numerator and denominator use the same values so
           the approximation error largely cancels in the softmax ratio.
  avT  = v'^T @ pT          -> [128, 512] PSUM per (head, Sq-chunk), with
           the matmul stream lag-emitted behind the exp stream.
  outT = av[64:128] * approx(1/av[0:64])  (DVE)
  y    = w_projA^T @ st01 + w_projB^T @ outT2 fused into ONE PSUM
           accumulation group per output chunk (single DRAM output, half
           the store traffic of separate yA/yB), written out in the SBUF
           staging layout so every store is one wide-descriptor DMA.  All
           y work sits in the tail: h2's AV chunks nq2/3 lag naturally in
           PSUM, nq0/1 stage sk0..14 through fp16 SBUF partials that an
           identity matmul re-injects into PSUM on top of the sk15 block,
           so every chunk finishes with a plain recip+mul on the DVE,
           interleaved between the y chunks' wpA/wpB matmul halves.
"""

import threading

import numpy as np

import concourse.bass as bass
import concourse.tile as tile
from concourse import bacc, mybir
from concourse.bass import ts, ds
from concourse.bass_utils import run_bass_kernel_spmd

F32 = mybir.dt.float32
F16 = mybir.dt.float16
I16 = mybir.dt.int16

EMBED = 768
NH = 12
D = 64
B = 2
S = 2048
HPC = 3          # heads per core
NCORES = 8
P = 128
KC = EMBED // P  # 6 contraction chunks
NQ = S // 512    # 4 Sq chunks of 512
NSK = S // P     # 16 Sk blocks
NT = 5           # qk projection tiles

# fp16 Schraudolph exp: J = s_true * 1024*log2(e) + (15360 + DELTA);
# bitcast(int16(J)) ~= e^s.  The 1024*log2(e)/8 factor is folded into the
# host-side q weights/biases so the scores PSUM is already in J units.
JSCALE = 1024.0 * np.log2(np.e)          # 1477.3195
JBIAS = 15360.0 - 45.0                   # delta=-45 balances the sawtooth
ACT_SCALE = float(1.0 / JSCALE)          # ScalarE exp reads the same psum

N_WARMUP = 26    # dummy matmuls that ramp the PE clock during the DMA-in


def _build_kernel(nc):
    # weights arrive host-packed [128, n] so each is one wide-descriptor DMA
    xT = nc.dram_tensor("xT", [EMBED, S], F16, kind="ExternalInput").ap()
    wqk = nc.dram_tensor("w_qk", [P, KC * 3 * P], F16, kind="ExternalInput").ap()
    b2d = nc.dram_tensor("b2", [P, 2], F32, kind="ExternalInput").ap()
    wv = nc.dram_tensor("w_v", [P, KC * HPC * D], F16, kind="ExternalInput").ap()
    wp = nc.dram_tensor("w_p", [P, 2 * EMBED], F16, kind="ExternalInput").ap()
    idn = nc.dram_tensor("ident", [P, P], F16, kind="ExternalInput").ap()
    # y stored in staging layout [nq*3+mt2, 128, 1024]: each drained SBUF
    # tile DMAs out as one contiguous-per-partition block (host reshuffles)
    y = nc.dram_tensor("y", [NQ * 3, P, 1024], F16, kind="ExternalOutput").ap()

    with tile.TileContext(nc) as tc:
        with (
            tc.tile_pool(name="wpool", bufs=1) as wpool,
            tc.tile_pool(name="xpool", bufs=1) as xpool,
            tc.tile_pool(name="qkpool", bufs=1) as qkpool,
            tc.tile_pool(name="vpool", bufs=1) as vpool,
            tc.tile_pool(name="ptpool", bufs=24) as ptpool,
            tc.tile_pool(name="opool", bufs=1) as opool,
            tc.tile_pool(name="rlpool", bufs=4) as rlpool,
            tc.tile_pool(name="psum", bufs=3, space="PSUM") as psum,
        ):
            # ---- PE warmup: dummy matmuls keep the PE busy (and its clock
            # ramping) from its ~7.5us engine-init floor while the DMAs
            # stream.
            dummy = wpool.tile([P, 512], F16, name="dummy")
            nc.vector.memset(dummy, 0.0)
            wps = psum.tile([P, 512], F32, tag="av", bufs=2, name="ps_warm")
            for _ in range(N_WARMUP):
                nc.tensor.matmul(wps, lhsT=dummy[:, 0:P], rhs=dummy,
                                 start=True, stop=True)

            # ---- loads.  The weights come host-packed as [128, n] tiles so
            # each is ONE dma_start with wide descriptors; the xT tiles are
            # row-split in two and the dma_starts spread across three
            # issuing engines (descriptor generation costs ~0.6us of the
            # issuing engine per dma_start, so one engine issuing them all
            # would serialize the stream).
            wqk_big = wpool.tile([P, KC * 3 * P], F16, name="wqk")
            nc.sync.dma_start(out=wqk_big[0:64, :], in_=wqk[0:64, :])
            nc.scalar.dma_start(out=wqk_big[64:P, :], in_=wqk[64:P, :])
            wqk_t = [wqk_big[:, ds(k * 3 * P, 3 * P)] for k in range(KC)]
            xT_t = []
            dma_engs = [nc.sync, nc.gpsimd, nc.scalar]
            for k in range(KC):
                xT_k = xpool.tile([P, S], F16, name=f"xT{k}")
                for r in range(2):
                    dma_engs[(2 * k + r) % 3].dma_start(
                        out=xT_k[ts(r, 64), :], in_=xT[ds(k * P + r * 64, 64), :]
                    )
                xT_t.append(xT_k)
            b2 = wpool.tile([P, 2], F32, name="b2")
            nc.sync.dma_start(out=b2, in_=b2d)
            wv_big = wpool.tile([P, KC * HPC * D], F16, name="wv")
            nc.gpsimd.dma_start(out=wv_big, in_=wv)
            wv_t = [wv_big[:, ds(k * HPC * D, HPC * D)] for k in range(KC)]
            wp_big = wpool.tile([P, 2 * EMBED], F16, name="wp")
            nc.sync.dma_start(out=wp_big, in_=wp)
            wpA = wp_big[:, 0:EMBED]
            wpB = wp_big[:, EMBED : 2 * EMBED]
            ident = wpool.tile([P, P], F16, name="ident")
            nc.gpsimd.dma_start(out=ident, in_=idn)

            # ---- persistent sbuf tensors + memsets (gpsimd, idle early)
            qkT = {
                t: qkpool.tile([P, S], F16, name=f"qkT{t}", tag=f"qkT{t}")
                for t in range(NT)
            }
            for t in (1, 2, 3, 4):
                zero = slice(D, P) if t == 1 else slice(0, D)
                nc.gpsimd.memset(qkT[t][zero, :], 0.0)
            vp = []
            for h in range(HPC):
                vp_h = vpool.tile([P, NSK * P], F16, name=f"vp{h}", tag=f"vp{h}")
                nc.gpsimd.memset(
                    vp_h.rearrange("p (s c) -> p s c", c=P)[:, :, 0:D], 1.0
                )
                vp.append(vp_h)
            st01 = opool.tile([P, S], F16)   # heads 0 (rows 0:64) and 1 (64:128)
            outT2 = opool.tile([P, S], F16)  # head 2 (rows 0:64; 64:128 zeroed)
            nc.gpsimd.memset(outT2[D:P, :], 0.0)

            # warm the Exp table while the loads stream
            wexp = rlpool.tile([P, 1], F32, name="wexp", tag="wexp", bufs=1)
            nc.scalar.activation(
                out=wexp, in_=dummy[:, 0:1],
                func=mybir.ActivationFunctionType.Exp, scale=1.0,
            )

            # ---- early qk projection: 8 partial-K PSUM groups fed as the
            # xT tiles land; left-half (Sq 0:1024) groups emitted first so
            # they only depend on the half-0 DMAs.
            scA = psum.tile([P, 1024], F32, tag="sc", name="scA")
            scB = psum.tile([P, 1024], F32, tag="sc", name="scB")
            scC = psum.tile([P, 1024], F32, tag="sc", name="scC")
            avA = psum.tile([P, 512], F32, tag="av", bufs=2, name="avA")
            avB = psum.tile([P, 512], F32, tag="av", bufs=2, name="avB")
            egroups = [
                (scA[:, 0:512], 0, 0),
                (scA[:, 512:1024], 0, 1),
                (scC[:, 0:512], 1, 0),
                (scC[:, 512:1024], 1, 1),
                (scB[:, 0:512], 0, 2),
                (scB[:, 512:1024], 0, 3),
                (avA, 1, 2),
                (avB, 1, 3),
            ]
            for k in range(KC):
                for dst, t, nq in egroups:
                    nc.tensor.matmul(
                        dst,
                        lhsT=wqk_t[k][:, ts(t, P)],
                        rhs=xT_t[k][:, ts(nq, 512)],
                        start=(k == 0),
                        stop=(k == KC - 1),
                    )
            # drains: the h0-critical pieces (qkT0, qkT1) go on the DVE, the
            # h1-only qkT2 copies on ScalarE, so neither engine runs 6 deep
            # when the first g1 exps arrive.
            nc.scalar.activation(
                out=qkT[0][:, 0:1024], in_=scA,
                func=mybir.ActivationFunctionType.Identity, bias=b2[:, 0:1],
            )
            nc.vector.tensor_copy(out=qkT[1][0:D, 0:1024], in_=scC[0:D, :])
            nc.scalar.activation(
                out=qkT[0][:, 1024:2048], in_=scB,
                func=mybir.ActivationFunctionType.Identity, bias=b2[:, 0:1],
            )
            nc.vector.tensor_copy(out=qkT[1][0:D, ts(2, 512)], in_=avA[0:D, :])
            nc.vector.tensor_copy(out=qkT[1][0:D, ts(3, 512)], in_=avB[0:D, :])
            nc.scalar.copy(out=qkT[2][D:P, 0:1024], in_=scC[D:P, :])
            nc.scalar.copy(out=qkT[2][D:P, ts(2, 512)], in_=avA[D:P, :])
            nc.scalar.copy(out=qkT[2][D:P, ts(3, 512)], in_=avB[D:P, :])

            # ---- kernel building blocks
            def scores_step(kt, qt, pts, act_both=False):
                """One Sk block of scores for one head: 4 matmuls + 2 exps,
                one on ScalarE (table) and one on VectorE (Schraudolph).
                act_both puts both on ScalarE — used for the last h2 steps
                so the DVE enters the tail with an empty queue."""
                sk = len(pts)
                pt = ptpool.tile([P, S], F16, name="pt", tag="pt")
                pts.append(pt)
                for g in range(2):
                    sps = psum.tile([P, 1024], F32, tag="sc", name="ps_s")
                    for j in range(2):
                        nc.tensor.matmul(
                            sps[:, ts(j, 512)],
                            lhsT=kt[:, ts(sk, P)],
                            rhs=qt[:, ds(g * 1024 + j * 512, 512)],
                            start=True,
                            stop=True,
                        )
                    if g == 1 and not act_both:
                        nc.vector.tensor_scalar_add(
                            out=pt[:, ts(g, 1024)].bitcast(I16),
                            in0=sps,
                            scalar1=JBIAS,
                        )
                    else:
                        nc.scalar.activation(
                            out=pt[:, ts(g, 1024)],
                            in_=sps,
                            func=mybir.ActivationFunctionType.Exp,
                            scale=ACT_SCALE,
                        )

            def qk2_chunk(n01):
                """tile2 = [q2|k2] projection for Sq chunks n01, n01+1:
                q2 -> qkT3 rows 64:128, k2 -> qkT4 rows 64:128."""
                sc = psum.tile([P, 1024], F32, tag="sc", name="ps_qk2")
                for k in range(KC):
                    for j in range(2):
                        nc.tensor.matmul(
                            sc[:, ts(j, 512)],
                            lhsT=wqk_t[k][:, ts(2, P)],
                            rhs=xT_t[k][:, ts(n01 + j, 512)],
                            start=(k == 0),
                            stop=(k == KC - 1),
                        )
                nc.vector.tensor_scalar_add(
                    out=qkT[3][D:P, ds(n01 * 512, 1024)],
                    in0=sc[0:D, :],
                    scalar1=b2[D:P, 1:2],
                )
                nc.scalar.copy(
                    out=qkT[4][D:P, ds(n01 * 512, 1024)], in_=sc[D:P, :]
                )

            def v_super(g4):
                """v projection for Sk blocks 4*g4 .. 4*g4+3 in one sc slot;
                one strided drain copy per head (ScalarE: DVE is on exp duty)."""
                # 256-col pitch per group: a 192-wide accumulation group must
                # not straddle a 512-float PSUM bank boundary
                vps = psum.tile([P, 1024], F32, tag="sc", name="ps_v")
                for j in range(4):
                    st = 4 * g4 + j
                    for k in range(KC):
                        nc.tensor.matmul(
                            vps[:, ds(j * 256, HPC * D)],
                            lhsT=xT_t[k][:, ts(st, P)],
                            rhs=wv_t[k],
                            start=(k == 0),
                            stop=(k == KC - 1),
                        )
                vv = vps.rearrange("p (j r) -> p j r", r=256)
                for h in range(HPC):
                    nc.scalar.copy(
                        out=vp[h].rearrange("p (s c) -> p s c", c=P)[
                            :, ds(4 * g4, 4), D:P
                        ],
                        in_=vv[:, :, ds(h * D, D)],
                    )

            def norm_chunk(h, nq, avp):
                rr = rlpool.tile([D, 512], F32, name="rr", tag="rr")
                nc.vector.reciprocal_approx_fast(out=rr, in_=avp[0:D, :])
                if h == 0:
                    dst = st01[0:D, ts(nq, 512)]
                elif h == 1:
                    dst = st01[D:P, ts(nq, 512)]
                else:
                    dst = outT2[0:D, ts(nq, 512)]
                nc.vector.tensor_mul(out=dst, in0=avp[D:P, :], in1=rr)

            class AvChunk:
                """Lag-emitted AV accumulation: emit() is called once per
                scores step so the matmul stream trails the exp stream."""

                def __init__(self, h, pts, nq, nsk, finish):
                    self.h, self.pts, self.nq, self.nsk, self.finish = (
                        h, pts, nq, nsk, finish)
                    self.av = psum.tile([P, 512], F32, tag="av", bufs=2, name="ps_av")
                    self.j = 0

                def emit(self, upto, cap=NSK):
                    upto = min(upto, self.j + cap)
                    while self.j < min(upto, self.nsk):
                        j = self.j
                        nc.tensor.matmul(
                            self.av,
                            lhsT=vp[self.h][:, ts(j, P)],
                            rhs=self.pts[j][:, ts(self.nq, 512)],
                            start=(j == 0),
                            stop=(j == self.nsk - 1),
                        )
                        self.j += 1
                    if self.j == self.nsk:
                        self.j += 1
                        self.finish(self.av)

            def av_burst(h, pts, nq):
                c = AvChunk(h, pts, nq, NSK, lambda av: norm_chunk(h, nq, av))
                c.emit(NSK)

            a2sb = {}

            def av2a_stage(nq, av):
                # fp16 staging via ScalarE: the tail re-injects it into PSUM
                # with an identity matmul, so no DVE adds are needed
                sb = rlpool.tile([P, 512], F16, name="a2s", tag="a2s", bufs=2)
                nc.scalar.copy(out=sb, in_=av)
                a2sb[nq] = sb

            def y_a(nq):
                """wpA half of the fused output projection: depends only on
                st01, so it can be emitted while the DVE still normalizes
                the outT2 chunk the wpB half needs."""
                tiles = [psum.tile([P, 1024], F32, tag="sc", name="ps_y")
                         for _ in range(3)]
                for mt2 in range(3):
                    for j in range(2):
                        nc.tensor.matmul(
                            tiles[mt2][:, ts(j, 512)],
                            lhsT=wpA[:, ts(2 * mt2 + j, P)],
                            rhs=st01[:, ts(nq, 512)],
                            start=True,
                            stop=False,
                        )
                return tiles

            def y_b(nq, tiles, last=False):
                """wpB half, tile-by-tile with drain+DMA interleaved.  The
                very last tile of the kernel drains as two halves on both
                engines with two DMA queues, shortening the final flush."""
                for mt2 in range(3):
                    for j in range(2):
                        nc.tensor.matmul(
                            tiles[mt2][:, ts(j, 512)],
                            lhsT=wpB[:, ts(2 * mt2 + j, P)],
                            rhs=outT2[:, ts(nq, 512)],
                            start=False,
                            stop=True,
                        )
                    ysb = rlpool.tile([P, 1024], F16, name="ysb", tag="ysb", bufs=4)
                    i = nq * 3 + mt2
                    if mt2 == 1:
                        nc.vector.tensor_copy(out=ysb, in_=tiles[mt2])
                    else:
                        nc.scalar.copy(out=ysb, in_=tiles[mt2])
                    (nc.gpsimd if mt2 % 2 else nc.sync).dma_start(out=y[i], in_=ysb)

            def y_chunk(nq, last=False):
                y_b(nq, y_a(nq), last)

            # ================= emission schedule =================
            pts0, pts1, pts2 = [], [], []

            # ---- h0 scores; fillers: remaining qk/v chunks + lagged av0
            lagged = []
            fill0 = {
                1: lambda: v_super(0),
                3: lambda: qk2_chunk(0),
                5: lambda: v_super(1),
                7: lambda: qk2_chunk(2),
                9: lambda: v_super(2),
                11: lambda: v_super(3),
            }
            for sk in range(NSK):
                scores_step(qkT[1], qkT[0], pts0)
                if sk in fill0:
                    fill0[sk]()
                if sk == 2:
                    lagged.append(AvChunk(0, pts0, 0, NSK,
                                          lambda av: norm_chunk(0, 0, av)))
                if sk == 4:
                    lagged.append(AvChunk(0, pts0, 1, NSK,
                                          lambda av: norm_chunk(0, 1, av)))
                for c in lagged:
                    c.emit(sk - 1)

            # ---- h1 scores; fillers: av0 tail bursts + lagged av1
            fast = []
            for sk in range(NSK):
                scores_step(qkT[2], qkT[0], pts1)
                if sk == 0:
                    for c in lagged:
                        c.emit(NSK)
                    lagged = []
                    fast.append(AvChunk(0, pts0, 2, NSK,
                                        lambda av: norm_chunk(0, 2, av)))
                if sk == 4:
                    fast.append(AvChunk(0, pts0, 3, NSK,
                                        lambda av: norm_chunk(0, 3, av)))
                for c in fast:
                    c.emit(NSK, cap=4)
                if sk == 2:
                    lagged.append(AvChunk(1, pts1, 0, NSK,
                                          lambda av: norm_chunk(1, 0, av)))
                if sk == 4:
                    lagged.append(AvChunk(1, pts1, 1, NSK,
                                          lambda av: norm_chunk(1, 1, av)))
                for c in lagged:
                    c.emit(sk - 1)

            # ---- h2 scores; fillers: av1 tail bursts, staged av2 (nq0/1
            # through SBUF partials) and natural-lag av2 chunks (nq2/3)
            fast = []
            av23 = []
            for sk in range(NSK):
                scores_step(qkT[4], qkT[3], pts2, act_both=(sk >= NSK - 2))
                if sk == 0:
                    for c in lagged:
                        c.emit(NSK)
                    lagged = []
                    fast.append(AvChunk(1, pts1, 2, NSK,
                                        lambda av: norm_chunk(1, 2, av)))
                if sk == 4:
                    fast.append(AvChunk(1, pts1, 3, NSK,
                                        lambda av: norm_chunk(1, 3, av)))
                for c in fast:
                    c.emit(NSK, cap=4)
                if sk == 2:
                    lagged.append(AvChunk(2, pts2, 0, NSK - 1,
                                          lambda av: av2a_stage(0, av)))
                if sk == 4:
                    lagged.append(AvChunk(2, pts2, 1, NSK - 1,
                                          lambda av: av2a_stage(1, av)))
                if sk == 8:
                    av23.append(AvChunk(2, pts2, 2, NSK, None))
                    av23[-1].finish = lambda av, c=av23[-1]: setattr(c, "done", av)
                if sk == 10:
                    av23.append(AvChunk(2, pts2, 3, NSK, None))
                    av23[-1].finish = lambda av, c=av23[-1]: setattr(c, "done", av)
                for c in lagged:
                    c.emit(min(sk - 1, NSK - 3))
                for c in av23:
                    c.emit(min(sk - 1, NSK - 2), cap=4)
            for c in lagged:
                c.emit(NSK - 1)

            # ---- tail: nq2/3 finish with their sk15 matmul + normalize;
            # nq0/1 via the staged partial + one matmul + adds; the fused
            # y chunks run on the PE while the DVE normalizes ahead.
            def av2b_mm(nq):
                """Re-inject the fp16 staged partial into PSUM (identity
                matmul) and accumulate the sk15 block on top: the finish is
                then a plain recip+mul like the other chunks."""
                av2b = psum.tile([P, 512], F32, tag="av", bufs=2, name="ps_a2b")
                nc.tensor.matmul(
                    av2b, lhsT=ident, rhs=a2sb[nq], start=True, stop=False,
                )
                nc.tensor.matmul(
                    av2b,
                    lhsT=vp[2][:, ts(NSK - 1, P)],
                    rhs=pts2[NSK - 1][:, ts(nq, 512)],
                    start=False,
                    stop=True,
                )
                return av2b

            # Tail: y(2)'s wpA half first (needs only st01 — its PSUM slots
            # freed by the ScalarE exps of sk14/15), covering the final-exp
            # latency; then the sk15 AV matmuls and the DVE normalize chain
            # interleaved so each y chunk's outT2 lands just in time.
            t2 = y_a(2)
            av23[0].emit(NSK)                     # sk15 matmul, stashes psum
            av23[1].emit(NSK)
            norm_chunk(2, 2, av23[0].done)        # DVE: norm2
            a2b0 = av2b_mm(0)                     # reuses av23[0]'s slot
            norm_chunk(2, 3, av23[1].done)        # DVE: norm3
            y_b(2, t2)
            a2b1 = av2b_mm(1)
            norm_chunk(2, 0, a2b0)                # DVE: outT2 nq0 ready
            t3 = y_a(3)
            y_b(3, t3)
            norm_chunk(2, 1, a2b1)
            t0 = y_a(0)
            y_b(0, t0)
            y_chunk(1, last=True)
    return nc


_CACHE = threading.Lock(), {}


def _get_nc():
    lock, cache = _CACHE
    with lock:
        if "nc" not in cache:
            nc = bacc.Bacc("TRN2", target_bir_lowering=False, debug=False)
            _build_kernel(nc)
            nc.compile()
            cache["nc"] = nc
        return cache["nc"]


def _shard_inputs(x, w_qkv, b_qkv, w_proj):
    """Build the 8 per-core input maps (host-side sharding/layout)."""
    qscale = float(JSCALE) / 8.0   # fold the Schraudolph J scale into q
    in_maps = []
    for c in range(NCORES):
        b = c // 4
        hg = c % 4
        h0 = HPC * hg
        qc = [np.s_[D * (h0 + i) : D * (h0 + i + 1)] for i in range(HPC)]
        kc = [np.s_[EMBED + D * (h0 + i) : EMBED + D * (h0 + i + 1)] for i in range(HPC)]
        vc = [np.s_[2 * EMBED + D * (h0 + i) : 2 * EMBED + D * (h0 + i + 1)] for i in range(HPC)]

        # projected w tiles: [q0|q1], [k0|k1], [q2|k2]; q columns prescaled
        w_qk = np.zeros((EMBED, 3 * P), dtype=np.float32)
        halves = [
            (0, 0, qc[0], qscale), (0, 1, qc[1], qscale),
            (1, 0, kc[0], 1.0), (1, 1, kc[1], 1.0),
            (2, 0, qc[2], qscale), (2, 1, kc[2], 1.0),
        ]
        for t, half, cols, sc in halves:
            w_qk[:, t * P + half * D : t * P + half * D + D] = w_qkv[:, cols] * sc
        # q biases only: col 0 = [q0|q1], col 1 rows 64:128 = q2. The k
        # biases are constant over keys and cancel in the softmax; the v
        # bias is applied on the host.
        b2 = np.zeros((P, 2), dtype=np.float32)
        b2[0:D, 0] = b_qkv[qc[0]] * qscale
        b2[D:P, 0] = b_qkv[qc[1]] * qscale
        b2[D:P, 1] = b_qkv[qc[2]] * qscale

        w_v = np.concatenate([w_qkv[:, s] for s in vc], axis=1)
        # w_proj rows for these heads; B half zero-padded to K=128
        w_p = np.zeros((2 * P, EMBED), dtype=np.float32)
        w_p[0:P] = w_proj[D * h0 : D * h0 + P]
        w_p[P : P + D] = w_proj[D * h0 + P : D * (h0 + HPC)]
        # pack the [6*128, n] weight stacks as [128, 6*n] so each SBUF tile
        # loads with one wide-descriptor DMA
        w_qk_p = w_qk.reshape(KC, P, 3 * P).transpose(1, 0, 2).reshape(P, KC * 3 * P)
        w_v_p = w_v.reshape(KC, P, HPC * D).transpose(1, 0, 2).reshape(P, KC * HPC * D)
        w_p_p = w_p.reshape(2, P, EMBED).transpose(1, 0, 2).reshape(P, 2 * EMBED)
        in_maps.append(
            {
                "xT": np.ascontiguousarray(x[b].T).astype(np.float16),
                "w_qk": np.ascontiguousarray(w_qk_p).astype(np.float16),
                "b2": b2,
                "w_v": np.ascontiguousarray(w_v_p).astype(np.float16),
                "w_p": np.ascontiguousarray(w_p_p).astype(np.float16),
                "ident": np.eye(P, dtype=np.float16),
            }
        )
    return in_maps


def kernel(x, w_qkv, b_qkv, w_proj, b_proj, _results_hook=None):
    x = np.asarray(x, dtype=np.float32)
    w_qkv = np.asarray(w_qkv, dtype=np.float32)
    b_qkv = np.asarray(b_qkv, dtype=np.float32)
    w_proj = np.asarray(w_proj, dtype=np.float32)
    b_proj = np.asarray(b_proj, dtype=np.float32)

    nc = _get_nc()
    in_maps = _shard_inputs(x, w_qkv, b_qkv, w_proj)
    res = run_bass_kernel_spmd(nc, in_maps, core_ids=list(range(NCORES)))
    if _results_hook is not None:
        _results_hook(res)

    # unshard: sum the 4 head-group partials per batch, add bias terms.
    # y arrives in staging layout [nq*3+mt2, 128, 1024]; logical row
    # 2*mt2*128 + (c//512)*128 + p, col nq*512 + c%512.
    b_v = b_qkv[2 * EMBED :]
    bias_row = b_v @ w_proj + b_proj  # [768]
    out = np.empty((B, S, EMBED), dtype=np.float32)
    for b in range(B):
        acc = np.zeros((EMBED, S), dtype=np.float32)
        for hg in range(4):
            yst = res.results[4 * b + hg]["y"].astype(np.float32)
            yst = yst.reshape(NQ, 3, P, 2, 512)          # nq, mt2, p, half, c
            acc += (
                yst.transpose(1, 3, 2, 0, 4)             # mt2, half, p, nq, c
                .reshape(EMBED, S)
            )
        out[b] = acc.T + bias_row
    return out
